# revision 6
# baseline (speedup 1.0000x reference)
"""Trainium2 Bass kernel for nn_AttentionSpace_87729001988510.

Batched channel-attention: 3 depthwise convs (K=7) over L, score = QK^T over
L (contracting L), softmax over channels, out = attn @ V.

Sharding: data-parallel over batch B=8 across the 8 NeuronCores (one batch
element per core). Per-core program (C=1024, L=4096, P=128):

Engine plan (v2 — convs off the critical-path engines, balanced):
  PE:   k-convs ci0-7 and q-convs ci6,7 (7-tap diag matmuls, quarter tiles),
        then score chunks (qT stationary via JIT transpose-reads, resident kT
        moving), then out chunks (E^T stationary, streamed v moving).
        Emitted back-to-back to keep HAM warm.
  DVE:  q-convs ci0-5 and v-convs ci0-5,7 as fused scalar_tensor_tensor
        7-tap chains accumulating in place (fp16 2x mode via an odd-shifted
        shadow of x), fp16 casts for its own chains, softmax max/reciprocal.
  ACT:  fp16 casts for the PE conv path, conv-psum->fp16 copies,
        exp(S-max) with fused row-sum, Es transposes, v6 mul-taps,
        phase-C psum copies scaled by 1/rowsum (softmax normalize folded in).
  GP:   x DMA issue (SWDGE), odd-shadow casts, v6 add-taps, edge memsets.
  Sync: weight loads, knat SBUF->SBUF xbar transposes into resident kT,
        q spills + JIT qT transpose-reads, v spills, v streaming for
        phase C, out DMAs.

q weights are pre-scaled by 1/sqrt(C) on host so softmax needs no rescale;
the normalize (1/rowsum) is folded into the phase-C psum copies. Output is
written fp16 and cast to fp32 on host.
"""

import numpy as np

import concourse.bass as bass
import concourse.tile as tile
from concourse import bacc, mybir
from concourse.bass_utils import run_bass_kernel_spmd

B = 8
C = 1024
L = 4096
K = 7
P = 128

NCC = C // P  # 8 channel chunks
NLC = L // P  # 32 l chunks
LB = 512
NLB = L // LB  # 8
LH = 2048  # half length (DVE conv granularity)
HW = LH + 6  # half tile width with halo
QL = 1024  # quarter length (PE conv granularity)
QW = QL + 6

f32 = mybir.dt.float32
f16 = mybir.dt.float16
AF = mybir.ActivationFunctionType
ALU = mybir.AluOpType


def _build():
    nc = bacc.Bacc("TRN2", target_bir_lowering=False, debug=False)

    x_in = nc.dram_tensor("x", [C, L], f32, kind="ExternalInput").ap()
    dk_in = nc.dram_tensor("dk", [C, K * P], f16, kind="ExternalInput").ap()
    dq_in = nc.dram_tensor("dq", [C, K * P], f16, kind="ExternalInput").ap()
    wq_in = nc.dram_tensor("wq", [C, K], f32, kind="ExternalInput").ap()
    wv_in = nc.dram_tensor("wv", [C, K], f32, kind="ExternalInput").ap()
    out_dram = nc.dram_tensor("out", [C, L], f16, kind="ExternalOutput").ap()
    q_dram = nc.dram_tensor("q_spill", [C, L], f16).ap()
    v_dram = nc.dram_tensor("v_spill", [C, L], f16).ap()
    v_dram3 = v_dram.rearrange("(dj p) l -> p dj l", p=P)

    from contextlib import ExitStack

    with tile.TileContext(nc) as tc:
        with ExitStack() as stack:
            ep = stack.enter_context
            big = ep(tc.tile_pool(name="big", bufs=1))
            pxk = ep(tc.tile_pool(name="pxk", bufs=3))  # f32 quarters, PE path
            pxqf = ep(tc.tile_pool(name="pxqf", bufs=2))  # f32 halves, DVE
            px67p = ep(tc.tile_pool(name="px67", bufs=2))  # f32 quarters, v6
            pxpk = ep(tc.tile_pool(name="pxpk", bufs=3))  # f16 quarters, PE
            pxq = ep(tc.tile_pool(name="pxq", bufs=2))  # f16 halves, DVE
            pnat = ep(tc.tile_pool(name="pnat", bufs=2))
            pqt = ep(tc.tile_pool(name="pqt", bufs=2))
            pes = ep(tc.tile_pool(name="pes", bufs=2))
            psm = ep(tc.tile_pool(name="psm", bufs=2))
            pv67 = ep(tc.tile_pool(name="pv67", bufs=2))
            pvc = ep(tc.tile_pool(name="pvc", bufs=2))
            post = ep(tc.tile_pool(name="post", bufs=4))
            pw = ep(tc.tile_pool(name="pw", bufs=2))
            pw6 = ep(tc.tile_pool(name="pw6", bufs=1))
            ps_c = ep(tc.tile_pool(name="ps_c", bufs=2, space="PSUM"))
            ps_s = ep(tc.tile_pool(name="ps_s", bufs=2, space="PSUM"))
            ps_o = ep(tc.tile_pool(name="ps_o", bufs=2, space="PSUM"))
            kT3 = big.tile([P, NLC, C], f16)
            ET3 = big.tile([P, NCC, C], f16)
            rcp_tiles = [big.tile([P, 1], f32, name=f"rcp{i}") for i in range(NCC)]

            # warm the exp table before it is needed
            warm = big.tile([P, 1], f32)
            nc.vector.memset(warm[:], 0.0)
            nc.scalar.activation(warm[:], warm[:], AF.Exp)

            qt_tiles = {}
            sps_tiles = {}

            # ---------- emission helpers ----------
            def load_x_tile(pool, tag, ci, base, width):
                """f32 tile whose col t = x[ci-chunk, base + t] (0-padded)."""
                px = pool.tile([P, width], f32, tag=tag)
                lo = max(base, 0)
                hi = min(base + width, L)
                if lo > base:
                    nc.gpsimd.memset(px[:, 0 : lo - base], 0.0)
                if hi < base + width:
                    nc.gpsimd.memset(px[:, hi - base :], 0.0)
                nc.gpsimd.dma_start(
                    px[:, lo - base : hi - base],
                    x_in[ci * P : (ci + 1) * P, lo:hi],
                )
                return px

            def pe_conv_block(ci, dmat_src, dst_dram):
                """PE 7-tap diag conv over 4 quarter tiles; psum copied to
                f16 on ACT, then spilled (q) or transposed into kT3 (k)."""
                dmat = pw.tile([P, K * P], f16, tag="dk")
                nc.sync.dma_start(dmat[:], dmat_src[ci * P : (ci + 1) * P, :])
                pxs = [
                    load_x_tile(pxk, "pxk", ci, qt * QL - 3, QW) for qt in range(4)
                ]
                xts = [None] * 4
                for qt in range(3):
                    xts[qt] = pxpk.tile([P, QW], f16, tag="xpk", name=f"xpk{qt}")
                    nc.scalar.copy(xts[qt][:], pxs[qt][:])
                for qt in range(4):
                    if qt == 1:
                        # defer the 4th cast so it doesn't block the ACT
                        # queue on the xpk ring while quarter 0 runs on PE
                        xts[3] = pxpk.tile([P, QW], f16, tag="xpk", name="xpk3")
                        nc.scalar.copy(xts[3][:], pxs[3][:])
                    nat = pnat.tile([P, QL], f16, tag="knat")
                    for lb in range(2):
                        ps = ps_c.tile([P, LB], f32, tag="cps")
                        for j in range(K):
                            nc.tensor.matmul(
                                ps[:],
                                dmat[:, j * P : (j + 1) * P],
                                xts[qt][:, lb * LB + j : lb * LB + j + LB],
                                start=(j == 0),
                                stop=(j == K - 1),
                            )
                        nc.scalar.copy(nat[:, lb * LB : (lb + 1) * LB], ps[:])
                    if dst_dram is None:
                        nc.sync.dma_start_transpose(
                            kT3[:, 8 * qt : 8 * (qt + 1), ci * P : (ci + 1) * P],
                            nat[:],
                        )
                    else:
                        nc.sync.dma_start(
                            dst_dram[ci * P : (ci + 1) * P, qt * QL : (qt + 1) * QL],
                            nat[:],
                        )

            def dve_conv_half(xq, xoq, w, dst):
                """Fused 7-tap chain on DVE, in-place accumulation into dst."""
                nc.vector.tensor_scalar_mul(dst, xq[:, 0:LH], w[:, 0:1])
                for j in range(1, K):
                    src = (
                        xq[:, j : j + LH]
                        if j % 2 == 0
                        else xoq[:, j - 1 : j - 1 + LH]
                    )
                    nc.vector.scalar_tensor_tensor(
                        dst, src, w[:, j : j + 1], dst, ALU.mult, ALU.add
                    )

            def dve_conv_block(ci, w_src, w_tag, dst_dram):
                w = pw.tile([P, K], f32, tag=w_tag)
                nc.sync.dma_start(w[:], w_src[ci * P : (ci + 1) * P, :])
                for h in range(2):
                    px = load_x_tile(pxqf, "pxq", ci, h * LH - 3, HW)
                    xq = pxq.tile([P, HW], f16, tag="xpq")
                    nc.vector.tensor_copy(xq[:], px[:])
                    xoq = pxq.tile([P, HW - 2], f16, tag="xpoq")
                    nc.gpsimd.tensor_copy(xoq[:], px[:, 1 : HW - 1])
                    nat = pnat.tile([P, LH], f16, tag="qnat")
                    dve_conv_half(xq, xoq, w, nat[:])
                    nc.sync.dma_start(
                        dst_dram[ci * P : (ci + 1) * P, h * LH : (h + 1) * LH],
                        nat[:],
                    )

            def qt_read(ci):
                qt = pqt.tile([P, NLC, P], f16, tag="qt")
                nc.sync.dma_start_transpose(qt[:], q_dram[ci * P : (ci + 1) * P, :])
                qt_tiles[ci] = qt

            def score_block(ci):
                if ci + 2 < NCC:
                    qt_read(ci + 2)
                sps = ps_s.tile([P, C], f32, tag="sps")
                qt = qt_tiles[ci]
                for lc in range(NLC):
                    for hb in range(2):
                        nc.tensor.matmul(
                            sps[:, hb * 512 : (hb + 1) * 512],
                            qt[:, lc, :],
                            kT3[:, lc, hb * 512 : (hb + 1) * 512],
                            start=(lc == 0),
                            stop=(lc == NLC - 1),
                        )
                sps_tiles[ci] = sps

            def sm_block(ci):
                sps = sps_tiles.pop(ci)
                m = psm.tile([P, 1], f32, tag="m")
                nc.vector.tensor_reduce(m[:], sps[:], mybir.AxisListType.X, ALU.max)
                mneg = psm.tile([P, 1], f32, tag="mneg")
                nc.vector.tensor_scalar_mul(mneg[:], m[:], -1.0)
                rs = psm.tile([P, 1], f32, tag="rs")
                es = pes.tile([P, C], f16, tag="es")
                nc.scalar.activation(
                    es[:], sps[:], AF.Exp, bias=mneg[:], accum_out=rs[:]
                )
                nc.vector.reciprocal(rcp_tiles[ci][:], rs[:])
                nc.scalar.dma_start_transpose(
                    ET3[:, :, ci * P : (ci + 1) * P], es[:]
                )

            def v6_quarter(ci, w6, qt):
                """v-conv quarter via ACT mul taps + GPSIMD add taps."""
                px = load_x_tile(px67p, "px67", ci, qt * QL - 3, QW)
                acc = pv67.tile([P, QL], f16, tag="v67a")
                t0 = pv67.tile([P, QL], f16, tag="v67t")
                nc.scalar.activation(t0[:], px[:, 0:QL], AF.Copy, scale=w6[:, 0:1])
                t1 = pv67.tile([P, QL], f16, tag="v67t")
                nc.scalar.activation(t1[:], px[:, 1 : 1 + QL], AF.Copy, scale=w6[:, 1:2])
                nc.gpsimd.tensor_tensor(acc[:], t0[:], t1[:], ALU.add)
                for j in range(2, K):
                    tj = pv67.tile([P, QL], f16, tag="v67t")
                    nc.scalar.activation(
                        tj[:], px[:, j : j + QL], AF.Copy, scale=w6[:, j : j + 1]
                    )
                    nc.gpsimd.tensor_tensor(acc[:], acc[:], tj[:], ALU.add)
                nc.sync.dma_start(
                    v_dram[ci * P : (ci + 1) * P, qt * QL : (qt + 1) * QL],
                    acc[:],
                )

            # ---------- emission ----------
            for ci in range(NCC):
                pe_conv_block(ci, dk_in, None)  # k -> kT3 resident
                if ci < 6:
                    dve_conv_block(ci, wq_in, "wq", q_dram)
                if ci == 1:
                    qt_read(0)
                    qt_read(1)

            score_block(0)
            pe_conv_block(6, dq_in, q_dram)
            sm_block(0)
            score_block(1)
            pe_conv_block(7, dq_in, q_dram)
            sm_block(1)
            for ci in range(2, NCC):
                score_block(ci)

            w6 = pw6.tile([P, K], f32, tag="wv6")
            nc.sync.dma_start(w6[:], wv_in[6 * P : 7 * P, :])
            vseq = [0, 1, 2, 3, 4, 5, 7]
            for i, vc_i in enumerate(vseq):
                if i < 4:
                    v6_quarter(6, w6, i)
                dve_conv_block(vc_i, wv_in, "wv", v_dram)
                if i + 2 < NCC:
                    sm_block(i + 2)

            # ---------- phase C: out = diag(rcp) E @ V ----------
            for lb in range(NLB):
                vc = pvc.tile([P, NCC, LB], f16, tag="vc")
                nc.sync.dma_start(vc[:], v_dram3[:, :, lb * LB : (lb + 1) * LB])
                for ci in range(NCC):
                    ops = ps_o.tile([P, LB], f32, tag="ops")
                    for dj in range(NCC):
                        nc.tensor.matmul(
                            ops[:],
                            ET3[:, dj, ci * P : (ci + 1) * P],
                            vc[:, dj, :],
                            start=(dj == 0),
                            stop=(dj == NCC - 1),
                        )
                    ost = post.tile([P, LB], f16, tag="ost")
                    nc.scalar.activation(
                        ost[:], ops[:], AF.Copy, scale=rcp_tiles[ci][:]
                    )
                    nc.sync.dma_start(
                        out_dram[ci * P : (ci + 1) * P, lb * LB : (lb + 1) * LB],
                        ost[:],
                    )

    nc.compile()
    return nc


_nc_cache = None


def _get_nc():
    global _nc_cache
    if _nc_cache is None:
        _nc_cache = _build()
    return _nc_cache


def _diag_blocks(w: np.ndarray) -> np.ndarray:
    """w: [C, 1, K] -> [C, K*P] f16 where row r, block j has diag entry at
    column j*P + (r % P) equal to w[r, 0, j]."""
    d = np.zeros((C, K * P), np.float16)
    r = np.arange(C)
    for j in range(K):
        d[r, j * P + (r % P)] = w[r, 0, j].astype(np.float16)
    return d


def _in_maps(x, q_w, k_w, v_w):
    x = np.ascontiguousarray(np.asarray(x, dtype=np.float32))
    isc = np.float32(1.0 / np.sqrt(C))
    q_w = np.asarray(q_w, dtype=np.float32)
    dk = _diag_blocks(np.asarray(k_w, dtype=np.float32))
    dq = _diag_blocks(q_w * isc)
    wq = np.ascontiguousarray(q_w[:, 0, :] * isc)
    wv = np.ascontiguousarray(np.asarray(v_w, dtype=np.float32)[:, 0, :])
    return [
        {"x": np.ascontiguousarray(x[b]), "dk": dk, "dq": dq, "wq": wq, "wv": wv}
        for b in range(B)
    ]


def kernel(x, q_w, k_w, v_w):
    nc = _get_nc()
    res = run_bass_kernel_spmd(nc, _in_maps(x, q_w, k_w, v_w), list(range(B)))
    out = np.stack([res.results[b]["out"] for b in range(B)]).astype(np.float32)
    return out


# revision 12
# speedup vs baseline: 1.4632x; 1.4632x over previous
"""Trainium2 Bass kernel for nn_AttentionSpace_87729001988510.

Batched channel-attention: 3 depthwise convs (K=7) over L, score = QK^T over
L (contracting L), softmax over channels, out = attn @ V.

Sharding: data-parallel over batch B=8 across the 8 NeuronCores (one batch
element per core). Per-core program (C=1024, L=4096, P=128):

v3 engine plan (costs calibrated on HW: DVE TS 4x / TT 2x; STT and CAST ops
are 1x so both are avoided; GPSIMD only issues DMA):
  x is cast fp32->fp16 once (ACT) on its way into the PE k-convs and the
  interior is spilled to x16_dram; every other conv path reloads fp16 tiles,
  including an odd-shifted shadow copy via a shifted DRAM load (keeps all
  DVE operands 4B-aligned for the 2x/4x modes).
  PE:   k-convs ci0-7, q-convs ci6,7, v-conv ci0 (7-tap diag matmuls),
        score chunks (qT stationary via JIT transpose-reads, resident kT
        moving), out chunks (E^T stationary, streamed v moving).
  DVE:  q-convs ci0-5 and v-convs ci1-3 as 7xTS + 6xTT chains (in-place
        accumulate), add-taps for the ACT-assisted v-convs ci4-7,
        softmax row-max + reciprocal.
  ACT:  fp16 casts for the PE path, conv-psum->fp16 copies, mul-taps for
        v-convs ci4-7, exp(S-max) with fused row-sum, Es transposes,
        phase-C psum copies scaled by 1/rowsum (softmax normalize).
  GP:   f32 x load issue, edge memsets.
  Sync: everything else DMA: weight loads, x16/q/v spills, fp16 reloads,
        knat SBUF->SBUF transposes into resident kT, JIT qT reads,
        v streaming for phase C, out stores.
"""

import numpy as np

import concourse.bass as bass
import concourse.tile as tile
from concourse import bacc, mybir
from concourse.bass_utils import run_bass_kernel_spmd

B = 8
C = 1024
L = 4096
K = 7
P = 128

NCC = C // P  # 8 channel chunks
NLC = L // P  # 32 l chunks
LB = 512
NLB = L // LB  # 8
LH = 2048  # half length
HW = LH + 6  # half tile width with halo

f32 = mybir.dt.float32
f16 = mybir.dt.float16
AF = mybir.ActivationFunctionType
ALU = mybir.AluOpType

PE_V = [0, 7]  # v-conv chunks on PE
DVE_V = [1, 2, 3]  # v-conv chunks on DVE
ACT_V = [4, 5, 6]  # v-conv chunks with ACT muls + deferred DVE adds


def _build():
    nc = bacc.Bacc("TRN2", target_bir_lowering=False, debug=False)

    x_in = nc.dram_tensor("x", [C, L], f32, kind="ExternalInput").ap()
    dk_in = nc.dram_tensor("dk", [C, K * P], f16, kind="ExternalInput").ap()
    dq_in = nc.dram_tensor("dq", [C, K * P], f16, kind="ExternalInput").ap()
    dv_in = nc.dram_tensor("dv", [C, K * P], f16, kind="ExternalInput").ap()
    wq_in = nc.dram_tensor("wq", [C, K], f32, kind="ExternalInput").ap()
    wv_in = nc.dram_tensor("wv", [C, K], f32, kind="ExternalInput").ap()
    out_dram = nc.dram_tensor("out", [C, L], f16, kind="ExternalOutput").ap()
    x16_dram = nc.dram_tensor("x16_spill", [C, L], f16).ap()
    q_dram = nc.dram_tensor("q_spill", [C, L], f16).ap()
    v_dram = nc.dram_tensor("v_spill", [C, L], f16).ap()
    v_dram3 = v_dram.rearrange("(dj p) l -> p dj l", p=P)

    from contextlib import ExitStack

    with tile.TileContext(nc) as tc:
        with ExitStack() as stack:
            ep = stack.enter_context
            big = ep(tc.tile_pool(name="big", bufs=1))
            pxf = ep(tc.tile_pool(name="pxf", bufs=2))  # f32 halves (initial)
            pxk = ep(tc.tile_pool(name="pxk", bufs=2))  # f16 halves, PE path
            pxq = ep(tc.tile_pool(name="pxq", bufs=2))  # f16 halves, DVE path
            ptmp = ep(tc.tile_pool(name="ptmp", bufs=2))  # DVE mul tmps
            patmp = ep(tc.tile_pool(name="patmp", bufs=8))  # ACT mul tmps
            pnat = ep(tc.tile_pool(name="pnat", bufs=2))
            pqt = ep(tc.tile_pool(name="pqt", bufs=2))
            pes = ep(tc.tile_pool(name="pes", bufs=2))
            psm = ep(tc.tile_pool(name="psm", bufs=2))
            pvc = ep(tc.tile_pool(name="pvc", bufs=2))
            post = ep(tc.tile_pool(name="post", bufs=3))
            pw = ep(tc.tile_pool(name="pw", bufs=2))
            ps_c = ep(tc.tile_pool(name="ps_c", bufs=2, space="PSUM"))
            ps_s = ep(tc.tile_pool(name="ps_s", bufs=2, space="PSUM"))
            ps_o = ep(tc.tile_pool(name="ps_o", bufs=2, space="PSUM"))

            kT3 = big.tile([P, NLC, C], f16)
            ET3 = big.tile([P, NCC, C], f16)
            rcp_tiles = [big.tile([P, 1], f32, name=f"rcp{i}") for i in range(NCC)]

            # warm the exp table before it is needed
            warm = big.tile([P, 1], f32)
            nc.vector.memset(warm[:], 0.0)
            nc.scalar.activation(warm[:], warm[:], AF.Exp)

            qt_tiles = {}
            sps_tiles = {}

            # ---------- emission helpers ----------
            def load_f16(pool, tag, name, ci, base, width):
                """f16 tile from x16_dram, col t = x[ci-chunk, base+t], padded."""
                xt = pool.tile([P, width], f16, tag=tag, name=name)
                lo = max(base, 0)
                hi = min(base + width, L)
                if lo > base:
                    nc.gpsimd.memset(xt[:, 0 : lo - base], 0.0)
                if hi < base + width:
                    nc.gpsimd.memset(xt[:, hi - base :], 0.0)
                nc.sync.dma_start(
                    xt[:, lo - base : hi - base],
                    x16_dram[ci * P : (ci + 1) * P, lo:hi],
                )
                return xt

            def pe_conv_quarters(dmat, src_half, h, nat):
                """Two 512-blocks of 7-tap diag conv from one f16 half tile."""
                for lb in range(4):
                    ps = ps_c.tile([P, LB], f32, tag="cps")
                    for j in range(K):
                        nc.tensor.matmul(
                            ps[:],
                            dmat[:, j * P : (j + 1) * P],
                            src_half[:, lb * LB + j : lb * LB + j + LB],
                            start=(j == 0),
                            stop=(j == K - 1),
                        )
                    nc.scalar.copy(nat[:, lb * LB : (lb + 1) * LB], ps[:])

            def k_block(ci):
                """x load + ACT cast (serves PE directly), x16 spill, k-conv,
                transpose into resident kT3."""
                dmat = pw.tile([P, K * P], f16, tag="dk")
                nc.sync.dma_start(dmat[:], dk_in[ci * P : (ci + 1) * P, :])
                for h in range(2):
                    px = pxf.tile([P, HW], f32, tag="pxf", name="pxf")
                    base = h * LH - 3
                    lo, hi = max(base, 0), min(base + HW, L)
                    if lo > base:
                        nc.gpsimd.memset(px[:, 0 : lo - base], 0.0)
                    if hi < base + HW:
                        nc.gpsimd.memset(px[:, hi - base :], 0.0)
                    nc.gpsimd.dma_start(
                        px[:, lo - base : hi - base],
                        x_in[ci * P : (ci + 1) * P, lo:hi],
                    )
                    xk = pxk.tile([P, HW], f16, tag="xpk", name="xpk")
                    nc.scalar.copy(xk[:], px[:])
                    nc.sync.dma_start(
                        x16_dram[ci * P : (ci + 1) * P, h * LH : (h + 1) * LH],
                        xk[:, 3 : 3 + LH],
                    )
                    nat = pnat.tile([P, LH], f16, tag="knat")
                    pe_conv_quarters(dmat, xk, h, nat[:])
                    nc.sync.dma_start_transpose(
                        kT3[:, 16 * h : 16 * (h + 1), ci * P : (ci + 1) * P],
                        nat[:],
                    )

            def pe_conv_block(ci, dmat_src, dst_dram):
                """PE diag conv for q6/q7/v0 from reloaded f16 halves."""
                dmat = pw.tile([P, K * P], f16, tag="dk", name="dmat2")
                nc.sync.dma_start(dmat[:], dmat_src[ci * P : (ci + 1) * P, :])
                for h in range(2):
                    xk = load_f16(pxk, "xpk", "xpk2", ci, h * LH - 3, HW)
                    nat = pnat.tile([P, LH], f16, tag="knat", name="nat2")
                    pe_conv_quarters(dmat, xk, h, nat[:])
                    nc.sync.dma_start(
                        dst_dram[ci * P : (ci + 1) * P, h * LH : (h + 1) * LH],
                        nat[:],
                    )

            def dve_chain_half(xq, xoq, w, dst):
                """7-tap conv: TS muls (4x) + TT adds (2x), acc in place."""
                nc.vector.tensor_scalar_mul(dst, xq[:, 0:LH], w[:, 0:1])
                for j in range(1, K):
                    src = (
                        xq[:, j : j + LH]
                        if j % 2 == 0
                        else xoq[:, j - 1 : j - 1 + LH]
                    )
                    tmp = ptmp.tile([P, LH], f16, tag="tmp")
                    nc.vector.tensor_scalar_mul(tmp[:], src, w[:, j : j + 1])
                    nc.vector.tensor_tensor(dst, dst, tmp[:], ALU.add)

            def dve_conv_block(ci, w_src, w_tag, dst_dram):
                w = pw.tile([P, K], f32, tag=w_tag, name="wdve")
                nc.sync.dma_start(w[:], w_src[ci * P : (ci + 1) * P, :])
                for h in range(2):
                    xq = load_f16(pxq, "xpq", "xpq", ci, h * LH - 3, HW)
                    xoq = load_f16(pxq, "xpoq", "xpoq", ci, h * LH - 2, HW - 2)
                    nat = pnat.tile([P, LH], f16, tag="qnat")
                    dve_chain_half(xq, xoq, w, nat[:])
                    nc.sync.dma_start(
                        dst_dram[ci * P : (ci + 1) * P, h * LH : (h + 1) * LH],
                        nat[:],
                    )

            QL = 1024

            def act_muls_quarter(ci, w, qt):
                """7 ACT mul-taps of a v-conv quarter into a tmp ring;
                returns the tmps for a later deferred DVE add pass."""
                xq = load_f16(pxq, "xpq", "xpqa", ci, qt * QL - 3, QL + 6)
                tmps = []
                for j in range(K):
                    tmp = patmp.tile([P, QL], f16, tag="atmp", name=f"atmp{j}")
                    nc.scalar.activation(
                        tmp[:], xq[:, j : j + QL], AF.Copy, scale=w[:, j : j + 1]
                    )
                    tmps.append(tmp)
                return tmps

            def act_adds_quarter(ci, qt, tmps):
                """Deferred DVE add pass for an ACT-mul quarter + spill."""
                nat = pnat.tile([P, QL], f16, tag="anat", name="anat")
                nc.vector.tensor_tensor(nat[:], tmps[0][:], tmps[1][:], ALU.add)
                for j in range(2, K):
                    nc.vector.tensor_tensor(nat[:], nat[:], tmps[j][:], ALU.add)
                nc.sync.dma_start(
                    v_dram[ci * P : (ci + 1) * P, qt * QL : (qt + 1) * QL], nat[:]
                )

            def qt_read(ci):
                qt = pqt.tile([P, NLC, P], f16, tag="qt")
                nc.sync.dma_start_transpose(qt[:], q_dram[ci * P : (ci + 1) * P, :])
                qt_tiles[ci] = qt

            def score_block(ci):
                if ci + 2 < NCC:
                    qt_read(ci + 2)
                sps = ps_s.tile([P, C], f32, tag="sps")
                qt = qt_tiles[ci]
                for lc in range(NLC):
                    for hb in range(2):
                        nc.tensor.matmul(
                            sps[:, hb * 512 : (hb + 1) * 512],
                            qt[:, lc, :],
                            kT3[:, lc, hb * 512 : (hb + 1) * 512],
                            start=(lc == 0),
                            stop=(lc == NLC - 1),
                        )
                sps_tiles[ci] = sps

            def sm_block(ci):
                sps = sps_tiles.pop(ci)
                m = psm.tile([P, 1], f32, tag="m")
                nc.vector.tensor_reduce(m[:], sps[:], mybir.AxisListType.X, ALU.max)
                mneg = psm.tile([P, 1], f32, tag="mneg")
                nc.vector.tensor_scalar_mul(mneg[:], m[:], -1.0)
                rs = psm.tile([P, 1], f32, tag="rs")
                es = pes.tile([P, C], f16, tag="es")
                nc.scalar.activation(
                    es[:], sps[:], AF.Exp, bias=mneg[:], accum_out=rs[:]
                )
                nc.vector.reciprocal(rcp_tiles[ci][:], rs[:])
                nc.scalar.dma_start_transpose(
                    ET3[:, :, ci * P : (ci + 1) * P], es[:]
                )

            # ---------- emission ----------
            for ci in range(NCC):
                k_block(ci)
                if ci < 6:
                    dve_conv_block(ci, wq_in, "wq", q_dram)
                if ci == 1:
                    qt_read(0)
                    qt_read(1)

            score_block(0)
            pe_conv_block(6, dq_in, q_dram)
            sm_block(0)
            score_block(1)
            pe_conv_block(7, dq_in, q_dram)
            sm_block(1)
            score_block(2)
            pe_conv_block(0, dv_in, v_dram)  # v0 on PE
            score_block(3)
            pe_conv_block(7, dv_in, v_dram)  # v7 on PE
            for ci in range(4, NCC):
                score_block(ci)

            # v-convs: ACT-mul quarters interleaved with DVE chain chunks;
            # the DVE add pass for each quarter is deferred one slot so the
            # DVE never stalls on in-flight ACT muls. sm blocks woven in.
            wv_tiles = {}
            for ci in ACT_V:
                wvt = big.tile([P, K], f32, name=f"wv{ci}")
                nc.sync.dma_start(wvt[:], wv_in[ci * P : (ci + 1) * P, :])
                wv_tiles[ci] = wvt

            aq = [(ci, qt) for ci in ACT_V for qt in range(4)]  # 12 quarters
            dve_work = [1, 2, 3]  # DVE chain chunks
            sm_seq = [2, 3, 4, 5, 6, 7]
            pend = None
            slots = []  # interleave pattern: mul-quarter / chain / sm
            for i in range(len(aq)):
                slots.append(("mul", aq[i]))
                if i < len(dve_work):
                    slots.append(("chain", dve_work[i]))
                if i < len(sm_seq):
                    slots.append(("sm", sm_seq[i]))
            for kind, arg in slots:
                if kind == "mul":
                    ci, qt = arg
                    if pend is not None:
                        act_adds_quarter(pend[0], pend[1], pend[2])
                    tmps = act_muls_quarter(ci, wv_tiles[ci], qt)
                    pend = (ci, qt, tmps)
                elif kind == "chain":
                    dve_conv_block(arg, wv_in, "wv", v_dram)
                else:
                    sm_block(arg)
            if pend is not None:
                act_adds_quarter(pend[0], pend[1], pend[2])

            # ---------- phase C: out = diag(rcp) E @ V ----------
            for lb in range(NLB):
                vc = pvc.tile([P, NCC, LB], f16, tag="vc")
                nc.sync.dma_start(vc[:], v_dram3[:, :, lb * LB : (lb + 1) * LB])
                for ci in range(NCC):
                    ops = ps_o.tile([P, LB], f32, tag="ops")
                    for dj in range(NCC):
                        nc.tensor.matmul(
                            ops[:],
                            ET3[:, dj, ci * P : (ci + 1) * P],
                            vc[:, dj, :],
                            start=(dj == 0),
                            stop=(dj == NCC - 1),
                        )
                    ost = post.tile([P, LB], f16, tag="ost")
                    nc.scalar.activation(
                        ost[:], ops[:], AF.Copy, scale=rcp_tiles[ci][:]
                    )
                    nc.sync.dma_start(
                        out_dram[ci * P : (ci + 1) * P, lb * LB : (lb + 1) * LB],
                        ost[:],
                    )

    nc.compile()
    return nc


_nc_cache = None


def _get_nc():
    global _nc_cache
    if _nc_cache is None:
        _nc_cache = _build()
    return _nc_cache


def _diag_blocks(w: np.ndarray) -> np.ndarray:
    """w: [C, 1, K] -> [C, K*P] f16 where row r, block j has diag entry at
    column j*P + (r % P) equal to w[r, 0, j]."""
    d = np.zeros((C, K * P), np.float16)
    r = np.arange(C)
    for j in range(K):
        d[r, j * P + (r % P)] = w[r, 0, j].astype(np.float16)
    return d


def _in_maps(x, q_w, k_w, v_w):
    x = np.ascontiguousarray(np.asarray(x, dtype=np.float32))
    isc = np.float32(1.0 / np.sqrt(C))
    q_w = np.asarray(q_w, dtype=np.float32)
    v_w = np.asarray(v_w, dtype=np.float32)
    dk = _diag_blocks(np.asarray(k_w, dtype=np.float32))
    dq = _diag_blocks(q_w * isc)
    dv = _diag_blocks(v_w)
    wq = np.ascontiguousarray(q_w[:, 0, :] * isc)
    wv = np.ascontiguousarray(v_w[:, 0, :])
    return [
        {
            "x": np.ascontiguousarray(x[b]),
            "dk": dk,
            "dq": dq,
            "dv": dv,
            "wq": wq,
            "wv": wv,
        }
        for b in range(B)
    ]


def kernel(x, q_w, k_w, v_w):
    nc = _get_nc()
    res = run_bass_kernel_spmd(nc, _in_maps(x, q_w, k_w, v_w), list(range(B)))
    out = np.stack([res.results[b]["out"] for b in range(B)]).astype(np.float32)
    return out
